# revision 22
# baseline (speedup 1.0000x reference)
"""Single-head causal attention forward on 8 TRN2 NeuronCores.

Problem: x [8, 2048, 1024] f32, Wq/Wk/Wv [128, 1024] f32.
  q/k/v = x @ W.T ; S = q k^T / sqrt(128) causal ; out = softmax(S) v.

Sharding: data-parallel, one batch element per core (8 cores).
Inside each core a flash-style blocked attention over 512-token chunks:
  - host pre-transposes x[b] -> xT [1024, 2048] so the contraction dim (c)
    lands on SBUF partitions with fully-contiguous DMA lines.
  - qT/kT/vT [h=128, t] via W-stationary matmuls (N=512, weight loads
    hidden); V natural [t, h] via 16 PE transposes of vT.
  - S^T[j, q] tiles: a ones-column appended to V makes the PV matmul also
    produce the softmax denominators (column sums of exp(S^T)), so no
    partition-direction reduction is ever needed.
  - exp on ScalarE with the 1/sqrt(128) scale folded into the activation;
    only the 16 diagonal 128x128 sub-blocks need a triangular 0/1 mask;
    strictly-masked blocks are never computed (causal skipping).
"""

import os
import sys

for _p in ("/opt/trn_rl_repo",):
    if _p not in sys.path and os.path.isdir(_p):
        sys.path.append(_p)

import numpy as np

B, T, D, H = 8, 2048, 1024, 128
CH = 512          # token chunk (free dim of S^T tiles)
NCH = T // CH     # 4 chunks
CC = D // 128     # 8 contraction sub-tiles
NT = T // 128     # 16 token tiles
SCALE = 1.0 / np.sqrt(np.float32(H))

# PROJ_DT: dtype of the QKV projection matmuls (x and W operands).
# ATT_DT: dtype of the attention matmuls (S^T and PV operands).
# float32 = 2 cyc/col, float32r = 1.5 (near-fp32 accuracy), bfloat16 = 1.
PROJ_DT = os.environ.get("KERNEL_PROJ_DT", "bfloat16")
ATT_DT = os.environ.get("KERNEL_ATT_DT", "bfloat16")

_CACHE = {}


def _build():
    import concourse.bacc as bacc
    import concourse.mybir as mybir
    import concourse.tile as tile

    dt = mybir.dt
    p_dt = getattr(dt, PROJ_DT)
    a_dt = getattr(dt, ATT_DT)

    nc = bacc.Bacc(None)
    xh = nc.declare_dram_parameter("xh", [NCH, 128, CC, CH], p_dt, isOutput=False)
    wqT = nc.declare_dram_parameter("wqT", [128, CC, H], p_dt, isOutput=False)
    wkT = nc.declare_dram_parameter("wkT", [128, CC, H], p_dt, isOutput=False)
    wvT = nc.declare_dram_parameter("wvT", [128, CC, H], p_dt, isOutput=False)
    tri = nc.declare_dram_parameter("tri", [128, 128], a_dt, isOutput=False)
    eye = nc.declare_dram_parameter("eye", [128, 128], a_dt, isOutput=False)
    out = nc.declare_dram_parameter("out", [T, H], dt.float32, isOutput=True)


    with tile.TileContext(nc) as tc:
        with (
            tc.tile_pool(name="singles", bufs=1) as singles,
            tc.tile_pool(name="xp", bufs=2) as xp,
            tc.tile_pool(name="qtp", bufs=2) as qtp,
            tc.tile_pool(name="ktp", bufs=4) as ktp,
            tc.tile_pool(name="vtp", bufs=2) as vtp,
            tc.tile_pool(name="ptp", bufs=16) as ptp,
            tc.tile_pool(name="outp", bufs=4) as outp,
            tc.tile_pool(name="recp", bufs=4) as recp,
            tc.tile_pool(name="psq", bufs=2, space="PSUM") as psq,
            tc.tile_pool(name="pss", bufs=3, space="PSUM") as pss,
            tc.tile_pool(name="pso", bufs=3, space="PSUM") as pso,
        ):
            # PE warmup: HAM releases the PE clock throttle (1.2->2.4 GHz)
            # only after ~3.4us of sustained activity. One accumulating
            # matmul chain over uninitialized tiles (values never read) has
            # no dependencies, so it runs during the DMA-wait window and
            # the real matmuls start at full clock.
            wu_a = singles.tile([128, 128], a_dt)
            wu_b = singles.tile([128, CH], a_dt)
            wu_a_ap, wu_b_ap = wu_a[:], wu_b[:]
            if ATT_DT == "float32r":
                wu_a_ap = wu_a_ap.bitcast(dt.float32)
                wu_b_ap = wu_b_ap.bitcast(dt.float32)
            nc.vector.memset(wu_a_ap, 0.0)
            nc.vector.memset(wu_b_ap, 0.0)
            wu_ps = pss.tile([128, CH], dt.float32, tag="sps")
            NWU = 10
            for i in range(NWU):
                nc.tensor.matmul(
                    wu_ps[:], wu_a[:], wu_b[:],
                    start=(i == 0), stop=(i == NWU - 1),
                )

            # --- constants / weights (loaded once) ---
            eye_sb = singles.tile([128, 128], a_dt)
            tri_sb = singles.tile([128, 128], a_dt)
            nc.gpsimd.dma_start(out=eye_sb[:], in_=eye[:])
            nc.gpsimd.dma_start(out=tri_sb[:], in_=tri[:])

            wq_sb = singles.tile([128, CC, H], p_dt)
            wk_sb = singles.tile([128, CC, H], p_dt)
            wv_sb = singles.tile([128, CC, H], p_dt)
            nc.scalar.dma_start(out=wq_sb[:], in_=wqT[:])
            nc.scalar.dma_start(out=wk_sb[:], in_=wkT[:])
            nc.scalar.dma_start(out=wv_sb[:], in_=wvT[:])
            # V' = [V | 1]; ones columns written once
            v_sb = singles.tile([128, NT, H + 4], a_dt)
            ones_ap = v_sb[:, :, H : H + 2]
            if ATT_DT == "float32r":
                ones_ap = ones_ap.bitcast(dt.float32)
            nc.vector.memset(ones_ap, 1.0)

            kt_tiles = []
            for qc in range(NCH):
                q0 = qc * CH
                # x chunk [128, CC, CH]; host layout keeps each piece a
                # single contiguous run per partition. Pieces alternate
                # between the two HWDGE queues (sync/scalar) for parallel
                # transfer; chunk 0 uses quarters so compute starts sooner.
                xt = xp.tile([128, CC, CH], p_dt)
                step = 2 if qc < 2 else 4
                for g0 in range(0, CC, step):
                    nc.sync.dma_start(
                        out=xt[:, g0 : g0 + step, :],
                        in_=xh[qc, :, g0 : g0 + step, :],
                    )

                # --- qT, kT, vT for this chunk: [h=128, CH] ---
                qps = psq.tile([128, CH], dt.float32, tag="qk")
                for cc in range(CC):
                    nc.tensor.matmul(
                        qps[:], wq_sb[:, cc, :], xt[:, cc, :],
                        start=(cc == 0), stop=(cc == CC - 1),
                    )
                qt = qtp.tile([128, CH], a_dt)
                nc.vector.tensor_copy(qt[:], qps[:])

                kps = psq.tile([128, CH], dt.float32, tag="qk")
                for cc in range(CC):
                    nc.tensor.matmul(
                        kps[:], wk_sb[:, cc, :], xt[:, cc, :],
                        start=(cc == 0), stop=(cc == CC - 1),
                    )
                kt = ktp.tile([128, CH], a_dt)
                nc.vector.tensor_copy(kt[:], kps[:])
                kt_tiles.append(kt)

                vps = psq.tile([128, CH], dt.float32, tag="qk")
                for cc in range(CC):
                    nc.tensor.matmul(
                        vps[:], wv_sb[:, cc, :], xt[:, cc, :],
                        start=(cc == 0), stop=(cc == CC - 1),
                    )
                vt = vtp.tile([128, CH], a_dt)
                nc.vector.tensor_copy(vt[:], vps[:])

                # V natural [t, h] via PE transpose of each 128x128 block
                for ti in range(4):
                    jt = qc * 4 + ti
                    vtr = pso.tile([128, 128], a_dt, tag="ovp")
                    nc.tensor.transpose(
                        vtr[:], vt[:, ti * 128 : (ti + 1) * 128], eye_sb[:]
                    )
                    nc.vector.tensor_copy(v_sb[:, jt, 0:H], vtr[:])

                # --- S^T tiles + exp + PV, interleaved per q tile ---
                # pts[jt] holds exp(S^T) for j-tile jt; the diagonal block's
                # masked 128x128 lives in a separate tile so PV matmuls on
                # later columns of pt don't wait on the DVE mask-multiply.
                pts = []
                for jt in range(qc * 4 + 4):
                    sps = pss.tile([128, CH], dt.float32)
                    kt_src = kt_tiles[jt // 4]
                    nc.tensor.matmul(
                        sps[:],
                        kt_src[:, (jt % 4) * 128 : (jt % 4 + 1) * 128],
                        qt[:],
                        start=True, stop=True,
                    )
                    pt = ptp.tile([128, CH], a_dt)
                    if jt < qc * 4:
                        nc.scalar.activation(
                            pt[:], sps[:], mybir.ActivationFunctionType.Exp,
                            scale=float(SCALE),
                        )
                        pts.append((pt, None))
                    else:
                        vstart = (jt - qc * 4) * 128
                        if vstart + 128 < CH:
                            nc.scalar.activation(
                                pt[:, vstart + 128 : CH],
                                sps[:, vstart + 128 : CH],
                                mybir.ActivationFunctionType.Exp,
                                scale=float(SCALE),
                            )
                        pd = ptp.tile([128, 128], a_dt, tag="pd")
                        nc.scalar.activation(
                            pd[:], sps[:, vstart : vstart + 128],
                            mybir.ActivationFunctionType.Exp,
                            scale=float(SCALE),
                        )
                        nc.vector.tensor_mul(pd[:], pd[:], tri_sb[:])
                        pts.append((pt, pd))

                    # PV for q tile ti becomes runnable once its diagonal
                    # block (jt == qi) exists
                    if jt >= qc * 4:
                        ti = jt - qc * 4
                        qi = jt
                        ops = pso.tile([128, H + 4], dt.float32, tag="ovp")
                        for j2 in range(qi + 1):
                            p_full, p_diag = pts[j2]
                            if j2 == qi:
                                lhs = p_diag[:]
                            else:
                                lhs = p_full[:, ti * 128 : (ti + 1) * 128]
                            nc.tensor.matmul(
                                ops[:, 0 : H + 2],
                                lhs,
                                v_sb[:, j2, 0 : H + 2],
                                start=(j2 == 0), stop=(j2 == qi),
                            )
                        rec = recp.tile([128, 1], dt.float32)
                        nc.vector.reciprocal(rec[:], ops[:, H : H + 1])
                        ob = outp.tile([128, H], dt.float32)
                        nc.vector.tensor_scalar_mul(ob[:], ops[:, 0:H], rec[:])
                        nc.sync.dma_start(
                            out=out[qi * 128 : (qi + 1) * 128, :], in_=ob[:]
                        )

    nc.compile()
    return nc


def _get_nc():
    if "nc" not in _CACHE:
        _CACHE["nc"] = _build()
    return _CACHE["nc"]


def _np_dt(name):
    if name == "bfloat16":
        import ml_dtypes

        return ml_dtypes.bfloat16
    return np.float32


def _in_maps(x, Wq, Wk, Wv):
    pdt = _np_dt(PROJ_DT)
    adt = _np_dt(ATT_DT)
    def _wprep(W):
        # W [H, D] -> [128p, CC, H] with per-partition-contiguous rows
        WT = np.asarray(W, dtype=np.float32).T.reshape(CC, 128, H)
        return np.ascontiguousarray(WT.transpose(1, 0, 2)).astype(pdt)

    wq, wk, wv = _wprep(Wq), _wprep(Wk), _wprep(Wv)
    tri = np.triu(np.ones((128, 128), dtype=np.float32)).astype(adt)
    eye = np.eye(128, dtype=np.float32).astype(adt)
    x = np.asarray(x, dtype=np.float32)
    maps = []
    for b in range(B):
        # [qc, p, cc, t]: per (qc, p) a contiguous CC*CH run
        xh = np.ascontiguousarray(
            x[b].T.reshape(CC, 128, NCH, CH).transpose(2, 1, 0, 3)
        ).astype(pdt)
        maps.append(
            {
                "xh": xh, "wqT": wq, "wkT": wk, "wvT": wv,
                "tri": tri, "eye": eye,
            }
        )
    return maps


def kernel(x, Wq, Wk, Wv):
    from concourse.bass_utils import run_bass_kernel_spmd

    nc = _get_nc()
    res = run_bass_kernel_spmd(nc, _in_maps(x, Wq, Wk, Wv), core_ids=list(range(B)))
    return np.stack([res.results[b]["out"] for b in range(B)]).astype(np.float32)


# revision 23
# speedup vs baseline: 1.0208x; 1.0208x over previous
"""Single-head causal attention forward on 8 TRN2 NeuronCores.

Problem: x [8, 2048, 1024] f32, Wq/Wk/Wv [128, 1024] f32.
  q/k/v = x @ W.T ; S = q k^T / sqrt(128) causal ; out = softmax(S) v.

Sharding: data-parallel, one batch element per core (8 cores).
Inside each core a flash-style blocked attention over 512-token chunks:
  - host pre-transposes x[b] -> xT [1024, 2048] so the contraction dim (c)
    lands on SBUF partitions with fully-contiguous DMA lines.
  - qT/kT/vT [h=128, t] via W-stationary matmuls (N=512, weight loads
    hidden); V natural [t, h] via 16 PE transposes of vT.
  - S^T[j, q] tiles: a ones-column appended to V makes the PV matmul also
    produce the softmax denominators (column sums of exp(S^T)), so no
    partition-direction reduction is ever needed.
  - exp on ScalarE with the 1/sqrt(128) scale folded into the activation;
    only the 16 diagonal 128x128 sub-blocks need a triangular 0/1 mask;
    strictly-masked blocks are never computed (causal skipping).
"""

import os
import sys

for _p in ("/opt/trn_rl_repo",):
    if _p not in sys.path and os.path.isdir(_p):
        sys.path.append(_p)

import numpy as np

B, T, D, H = 8, 2048, 1024, 128
CH = 512          # token chunk (free dim of S^T tiles)
NCH = T // CH     # 4 chunks
CC = D // 128     # 8 contraction sub-tiles
NT = T // 128     # 16 token tiles
SCALE = 1.0 / np.sqrt(np.float32(H))

# PROJ_DT: dtype of the QKV projection matmuls (x and W operands).
# ATT_DT: dtype of the attention matmuls (S^T and PV operands).
# float32 = 2 cyc/col, float32r = 1.5 (near-fp32 accuracy), bfloat16 = 1.
PROJ_DT = os.environ.get("KERNEL_PROJ_DT", "bfloat16")
ATT_DT = os.environ.get("KERNEL_ATT_DT", "bfloat16")

_CACHE = {}


def _build():
    import concourse.bacc as bacc
    import concourse.mybir as mybir
    import concourse.tile as tile

    dt = mybir.dt
    p_dt = getattr(dt, PROJ_DT)
    a_dt = getattr(dt, ATT_DT)

    nc = bacc.Bacc(None)
    xh = nc.declare_dram_parameter("xh", [NCH, 128, CC, CH], p_dt, isOutput=False)
    wqT = nc.declare_dram_parameter("wqT", [128, CC, H], p_dt, isOutput=False)
    wkT = nc.declare_dram_parameter("wkT", [128, CC, H], p_dt, isOutput=False)
    wvT = nc.declare_dram_parameter("wvT", [128, CC, H], p_dt, isOutput=False)
    tri = nc.declare_dram_parameter("tri", [128, 128], a_dt, isOutput=False)
    eye = nc.declare_dram_parameter("eye", [128, 128], a_dt, isOutput=False)
    out = nc.declare_dram_parameter("out", [T, H], dt.float32, isOutput=True)


    with tile.TileContext(nc) as tc:
        with (
            tc.tile_pool(name="singles", bufs=1) as singles,
            tc.tile_pool(name="xp", bufs=2) as xp,
            tc.tile_pool(name="qtp", bufs=2) as qtp,
            tc.tile_pool(name="ktp", bufs=4) as ktp,
            tc.tile_pool(name="vtp", bufs=2) as vtp,
            tc.tile_pool(name="ptp", bufs=16) as ptp,
            tc.tile_pool(name="outp", bufs=4) as outp,
            tc.tile_pool(name="recp", bufs=4) as recp,
            tc.tile_pool(name="psq", bufs=2, space="PSUM") as psq,
            tc.tile_pool(name="pss", bufs=3, space="PSUM") as pss,
            tc.tile_pool(name="pso", bufs=3, space="PSUM") as pso,
        ):
            # PE warmup: HAM releases the PE clock throttle (1.2->2.4 GHz)
            # only after ~3.4us of sustained activity. One accumulating
            # matmul chain over uninitialized tiles (values never read) has
            # no dependencies, so it runs during the DMA-wait window and
            # the real matmuls start at full clock.
            wu_a = singles.tile([128, 128], a_dt)
            wu_b = singles.tile([128, CH], a_dt)
            wu_a_ap, wu_b_ap = wu_a[:], wu_b[:]
            if ATT_DT == "float32r":
                wu_a_ap = wu_a_ap.bitcast(dt.float32)
                wu_b_ap = wu_b_ap.bitcast(dt.float32)
            nc.vector.memset(wu_a_ap, 0.0)
            nc.vector.memset(wu_b_ap, 0.0)
            wu_ps = pss.tile([128, CH], dt.float32, tag="sps")
            NWU = 10
            for i in range(NWU):
                nc.tensor.matmul(
                    wu_ps[:], wu_a[:], wu_b[:],
                    start=(i == 0), stop=(i == NWU - 1),
                )

            # --- constants / weights (loaded once) ---
            eye_sb = singles.tile([128, 128], a_dt)
            tri_sb = singles.tile([128, 128], a_dt)
            nc.gpsimd.dma_start(out=eye_sb[:], in_=eye[:])
            nc.gpsimd.dma_start(out=tri_sb[:], in_=tri[:])

            wq_sb = singles.tile([128, CC, H], p_dt)
            wk_sb = singles.tile([128, CC, H], p_dt)
            wv_sb = singles.tile([128, CC, H], p_dt)
            nc.scalar.dma_start(out=wq_sb[:], in_=wqT[:])
            nc.scalar.dma_start(out=wk_sb[:], in_=wkT[:])
            nc.scalar.dma_start(out=wv_sb[:], in_=wvT[:])
            # V' = [V | 1]; ones columns written once
            v_sb = singles.tile([128, NT, H + 4], a_dt)
            ones_ap = v_sb[:, :, H : H + 2]
            if ATT_DT == "float32r":
                ones_ap = ones_ap.bitcast(dt.float32)
            nc.vector.memset(ones_ap, 1.0)

            kt_tiles = []
            for qc in range(NCH):
                q0 = qc * CH
                # x chunk [128, CC, CH]; host layout keeps each piece a
                # single contiguous run per partition. Pieces alternate
                # between the two HWDGE queues (sync/scalar) for parallel
                # transfer; chunk 0 uses quarters so compute starts sooner.
                xt = xp.tile([128, CC, CH], p_dt)
                step = 2 if qc < 2 else 4
                for g0 in range(0, CC, step):
                    nc.sync.dma_start(
                        out=xt[:, g0 : g0 + step, :],
                        in_=xh[qc, :, g0 : g0 + step, :],
                    )

                # --- qT, kT, vT for this chunk: [h=128, CH] ---
                qps = psq.tile([128, CH], dt.float32, tag="qk")
                for cc in range(CC):
                    nc.tensor.matmul(
                        qps[:], wq_sb[:, cc, :], xt[:, cc, :],
                        start=(cc == 0), stop=(cc == CC - 1),
                    )
                qt = qtp.tile([128, CH], a_dt)
                nc.vector.tensor_copy(qt[:], qps[:])

                kps = psq.tile([128, CH], dt.float32, tag="qk")
                for cc in range(CC):
                    nc.tensor.matmul(
                        kps[:], wk_sb[:, cc, :], xt[:, cc, :],
                        start=(cc == 0), stop=(cc == CC - 1),
                    )
                kt = ktp.tile([128, CH], a_dt)
                nc.vector.tensor_copy(kt[:], kps[:])
                kt_tiles.append(kt)

                vps = psq.tile([128, CH], dt.float32, tag="qk")
                for cc in range(CC):
                    nc.tensor.matmul(
                        vps[:], wv_sb[:, cc, :], xt[:, cc, :],
                        start=(cc == 0), stop=(cc == CC - 1),
                    )
                vt = vtp.tile([128, CH], a_dt)
                nc.vector.tensor_copy(vt[:], vps[:])

                # V natural [t, h] via PE transpose of each 128x128 block
                for ti in range(4):
                    jt = qc * 4 + ti
                    vtr = pso.tile([128, 128], a_dt, tag="ovp")
                    nc.tensor.transpose(
                        vtr[:], vt[:, ti * 128 : (ti + 1) * 128], eye_sb[:]
                    )
                    nc.vector.tensor_copy(v_sb[:, jt, 0:H], vtr[:])

                # --- S^T tiles + exp + PV, interleaved per q tile ---
                # pts[jt] holds exp(S^T) for j-tile jt; the diagonal block's
                # masked 128x128 lives in a separate tile so PV matmuls on
                # later columns of pt don't wait on the DVE mask-multiply.
                pts = []
                for jt in range(qc * 4 + 4):
                    sps = pss.tile([128, CH], dt.float32)
                    kt_src = kt_tiles[jt // 4]
                    v0 = (jt - qc * 4) * 128 if jt >= qc * 4 else 0
                    nc.tensor.matmul(
                        sps[:, v0:CH],
                        kt_src[:, (jt % 4) * 128 : (jt % 4 + 1) * 128],
                        qt[:, v0:CH],
                        start=True, stop=True,
                    )
                    pt = ptp.tile([128, CH], a_dt)
                    if jt < qc * 4:
                        nc.scalar.activation(
                            pt[:], sps[:], mybir.ActivationFunctionType.Exp,
                            scale=float(SCALE),
                        )
                        pts.append((pt, None))
                    else:
                        vstart = (jt - qc * 4) * 128
                        nc.scalar.activation(
                            pt[:, vstart:CH], sps[:, vstart:CH],
                            mybir.ActivationFunctionType.Exp,
                            scale=float(SCALE),
                        )
                        # masked diagonal block out-of-place: PV matmuls on
                        # later columns of pt don't wait for the mask mul
                        pd = ptp.tile([128, 128], a_dt, tag="pd")
                        nc.vector.tensor_mul(
                            pd[:], pt[:, vstart : vstart + 128], tri_sb[:]
                        )
                        pts.append((pt, pd))

                    # PV for q tile ti becomes runnable once its diagonal
                    # block (jt == qi) exists
                    if jt >= qc * 4:
                        ti = jt - qc * 4
                        qi = jt
                        ops = pso.tile([128, H + 4], dt.float32, tag="ovp")
                        for j2 in range(qi + 1):
                            p_full, p_diag = pts[j2]
                            if j2 == qi:
                                lhs = p_diag[:]
                            else:
                                lhs = p_full[:, ti * 128 : (ti + 1) * 128]
                            nc.tensor.matmul(
                                ops[:, 0 : H + 2],
                                lhs,
                                v_sb[:, j2, 0 : H + 2],
                                start=(j2 == 0), stop=(j2 == qi),
                            )
                        rec = recp.tile([128, 1], dt.float32)
                        nc.vector.reciprocal(rec[:], ops[:, H : H + 1])
                        ob = outp.tile([128, H], dt.float32)
                        nc.vector.tensor_scalar_mul(ob[:], ops[:, 0:H], rec[:])
                        nc.sync.dma_start(
                            out=out[qi * 128 : (qi + 1) * 128, :], in_=ob[:]
                        )

    nc.compile()
    return nc


def _get_nc():
    if "nc" not in _CACHE:
        _CACHE["nc"] = _build()
    return _CACHE["nc"]


def _np_dt(name):
    if name == "bfloat16":
        import ml_dtypes

        return ml_dtypes.bfloat16
    return np.float32


def _in_maps(x, Wq, Wk, Wv):
    pdt = _np_dt(PROJ_DT)
    adt = _np_dt(ATT_DT)
    def _wprep(W):
        # W [H, D] -> [128p, CC, H] with per-partition-contiguous rows
        WT = np.asarray(W, dtype=np.float32).T.reshape(CC, 128, H)
        return np.ascontiguousarray(WT.transpose(1, 0, 2)).astype(pdt)

    wq, wk, wv = _wprep(Wq), _wprep(Wk), _wprep(Wv)
    tri = np.triu(np.ones((128, 128), dtype=np.float32)).astype(adt)
    eye = np.eye(128, dtype=np.float32).astype(adt)
    x = np.asarray(x, dtype=np.float32)
    maps = []
    for b in range(B):
        # [qc, p, cc, t]: per (qc, p) a contiguous CC*CH run
        xh = np.ascontiguousarray(
            x[b].T.reshape(CC, 128, NCH, CH).transpose(2, 1, 0, 3)
        ).astype(pdt)
        maps.append(
            {
                "xh": xh, "wqT": wq, "wkT": wk, "wvT": wv,
                "tri": tri, "eye": eye,
            }
        )
    return maps


def kernel(x, Wq, Wk, Wv):
    from concourse.bass_utils import run_bass_kernel_spmd

    nc = _get_nc()
    res = run_bass_kernel_spmd(nc, _in_maps(x, Wq, Wk, Wv), core_ids=list(range(B)))
    return np.stack([res.results[b]["out"] for b in range(B)]).astype(np.float32)
